# revision 1
# baseline (speedup 1.0000x reference)
"""DensityAwareChamferLoss Trainium2 kernel.

Strategy: 8 cores = (4 batches) x (2 NN directions). Each core runs an
identical SPMD program computing, for 8192 query points against 8192
candidate points, the argmin of squared euclidean distance:

  PE:  s = 2*q.c - |c|^2 via K=4 fp32r matmuls -> PSUM fp32
  ACT: d = |q|^2 - s  (scale=-1, per-partition bias), cast bf16 -> SBUF
  DVE: fused min-reduce over the [128, 8192] strip (tensor_scalar accum),
       then max_index to recover up to 8 positions matching the min.

Host: bf16 ties (~0.4% of rows) are resolved by recomputing that row's
distances in fp32; counts/weights/loss are O(N) numpy.
"""

import sys

if "/opt/trn_rl_repo" not in sys.path:
    sys.path.insert(0, "/opt/trn_rl_repo")

import numpy as np

B = 4
N = 8192
QT = N // 128  # query tiles per core
N_CORES = 8

_CACHE = {}


def _build():
    from contextlib import ExitStack

    import concourse.bacc as bacc
    import concourse.bass as bass
    import concourse.tile as tile
    from concourse import mybir

    f32 = mybir.dt.float32
    f32r = mybir.dt.float32r
    bf16 = mybir.dt.bfloat16
    u32 = mybir.dt.uint32

    nc = bacc.Bacc("TRN2", target_bir_lowering=False, debug=False)
    qt4 = nc.dram_tensor("qt4", [4, N], f32r, kind="ExternalInput")
    ct4 = nc.dram_tensor("ct4", [4, N], f32r, kind="ExternalInput")
    qsq = nc.dram_tensor("qsq", [128, QT], f32, kind="ExternalInput")
    out_idx = nc.dram_tensor("out_idx", [QT, 128, 8], u32, kind="ExternalOutput")

    with tile.TileContext(nc) as tc:
        with ExitStack() as ctx:
            const = ctx.enter_context(tc.tile_pool(name="const", bufs=1))
            strips = ctx.enter_context(tc.tile_pool(name="strip", bufs=2))
            psum = ctx.enter_context(tc.tile_pool(name="psum", bufs=2, space="PSUM"))
            small = ctx.enter_context(tc.tile_pool(name="small", bufs=4))

            qt4_s = const.tile([4, N], f32r)
            nc.sync.dma_start(qt4_s[:], qt4.ap())
            ct4_s = const.tile([4, N], f32r)
            nc.sync.dma_start(ct4_s[:], ct4.ap())
            qsq_s = const.tile([128, QT], f32)
            nc.sync.dma_start(qsq_s[:], qsq.ap())
            zeros8 = const.tile([128, 8], f32)
            nc.vector.memset(zeros8[:], 0.0)

            for t in range(QT):
                strip = strips.tile([128, N], bf16, tag="strip")
                for g in range(4):
                    ps = psum.tile([128, 2048], f32, tag="ps")
                    for j in range(4):
                        nc.tensor.matmul(
                            ps[:, j * 512 : (j + 1) * 512],
                            qt4_s[:, t * 128 : (t + 1) * 128],
                            ct4_s[:, g * 2048 + j * 512 : g * 2048 + (j + 1) * 512],
                            start=True,
                            stop=True,
                        )
                    # d = -s + |q|^2, cast to bf16
                    nc.scalar.activation(
                        strip[:, g * 2048 : (g + 1) * 2048],
                        ps[:],
                        mybir.ActivationFunctionType.Identity,
                        bias=qsq_s[:, t : t + 1],
                        scale=-1.0,
                    )
                # fused: rewrite strip in place (x+0) and min-reduce into dmin
                dmin = small.tile([128, 1], f32, tag="dmin")
                nc.vector.tensor_scalar(
                    out=strip[:],
                    in0=strip[:],
                    scalar1=0.0,
                    scalar2=None,
                    op0=mybir.AluOpType.add,
                    op1=mybir.AluOpType.min,
                    accum_out=dmin[:],
                )
                # broadcast dmin to [128, 8] bf16 via ACT (scale=0, bias=dmin)
                min8 = small.tile([128, 8], bf16, tag="min8")
                nc.scalar.activation(
                    min8[:],
                    zeros8[:],
                    mybir.ActivationFunctionType.Identity,
                    bias=dmin[:],
                    scale=0.0,
                )
                idx8 = small.tile([128, 8], u32, tag="idx8")
                nc.vector.max_index(idx8[:], min8[:], strip[:])
                nc.sync.dma_start(out_idx.ap()[t], idx8[:])

    nc.compile()
    return nc


def _prep_core_inputs(q, c):
    # q, c: [N, 3] float32
    qt4 = np.empty((4, N), np.float32)
    qt4[0:3] = q.T
    qt4[3] = 1.0
    ct4 = np.empty((4, N), np.float32)
    ct4[0:3] = 2.0 * c.T
    csq = np.sum(c.astype(np.float32) * c.astype(np.float32), axis=1)
    ct4[3] = -csq
    qsq_flat = np.sum(q.astype(np.float32) * q.astype(np.float32), axis=1)
    qsq = qsq_flat.reshape(QT, 128).T.copy()
    return {"qt4": qt4, "ct4": ct4, "qsq": qsq}


def _d_row_fp32(q_row, c_all):
    # reference-formula distances of one query row vs all candidates, fp32
    return (
        np.sum(q_row * q_row).astype(np.float32)
        + np.sum(c_all * c_all, axis=1)
        - 2.0 * (c_all @ q_row)
    ).astype(np.float32)


def _indices_from_out(idx8, q, c):
    # idx8: [QT, 128, 8] uint32 -> idx [N] with host tie fixup
    cand = idx8.reshape(N, 8)
    idx = cand[:, 0].astype(np.int64)
    ambiguous = np.where(cand[:, 1] != np.uint32(0xFFFFFFFF))[0]
    for r in ambiguous:
        d = _d_row_fp32(q[r], c)
        idx[r] = int(np.argmin(d))
    return idx


def _loss_one(q, c, idx):
    # mean(1 - exp(-d) * (1/(count+eps))) for one direction (frac terms = 1)
    d = np.sum((q - c[idx]) ** 2, axis=1).astype(np.float32)
    cnt = np.bincount(idx, minlength=N).astype(np.float32)
    w = np.float32(1.0) / (cnt[idx] + np.float32(1e-6))
    return np.mean(np.float32(1.0) - np.exp(-d) * w, dtype=np.float32)


def run_cores(in_maps, trace=False):
    from concourse.bass_utils import run_bass_kernel_spmd

    if "nc" not in _CACHE:
        _CACHE["nc"] = _build()
    nc = _CACHE["nc"]
    res = run_bass_kernel_spmd(
        nc, in_maps, core_ids=list(range(N_CORES)), trace=trace
    )
    return res


def kernel(gts, preds):
    gts = np.ascontiguousarray(np.asarray(gts, dtype=np.float32))
    preds = np.ascontiguousarray(np.asarray(preds, dtype=np.float32))

    qc = []  # per-core (q, c)
    for core in range(N_CORES):
        b, direction = core >> 1, core & 1
        if direction == 0:
            qc.append((gts[b], preds[b]))
        else:
            qc.append((preds[b], gts[b]))

    in_maps = [_prep_core_inputs(q, c) for (q, c) in qc]
    res = run_cores(in_maps)

    loss = np.zeros(B, np.float32)
    per_dir = {}
    for core in range(N_CORES):
        q, c = qc[core]
        idx = _indices_from_out(np.asarray(res.results[core]["out_idx"]), q, c)
        per_dir[core] = _loss_one(q, c, idx)
    for b in range(B):
        loss[b] = (per_dir[2 * b] + per_dir[2 * b + 1]) / np.float32(2.0)
    return loss


# revision 6
# speedup vs baseline: 1.9224x; 1.9224x over previous
"""DensityAwareChamferLoss Trainium2 kernel.

Strategy: 8 cores = (4 batches) x (2 NN directions). Each core runs an
identical SPMD program computing, for 8192 query points against 8192
candidate points, the argmin of squared euclidean distance:

  PE:  s = 2*q.c - |c|^2 via K=4 float32 matmuls -> PSUM fp32
       (fp32, not fp32r: fp32r is tf32-grade and flips ~1.8% of argmins)
  ACT: d = |q|^2 - s  (scale=-1, per-partition bias), cast bf16 -> SBUF
  DVE: fused min-reduce over the [128, 8192] strip (tensor_scalar accum),
       then max_index to recover up to 8 positions matching the min.

Host: bf16 ties (~0.4% of rows) are resolved by recomputing that row's
distances in fp32 (exactly reproduces the reference argmin: 0 flips
measured on hardware); counts/weights/loss are O(N) numpy.

Engine budget per core (HW-calibrated cost model): PE 874us (bound),
ACT ~580us, DVE ~690us, total ~906us with overlap.
"""

import sys

if "/opt/trn_rl_repo" not in sys.path:
    sys.path.insert(0, "/opt/trn_rl_repo")

import numpy as np

B = 4
N = 8192
QT = N // 128  # query tiles per core
N_CORES = 8

_CACHE = {}


def _build(mm_dtype="float32", do_accum=True, do_argidx=True, reps=1,
           strip_bufs=2, psum_bufs=2, small_bufs=4):
    from contextlib import ExitStack

    import concourse.bacc as bacc
    import concourse.bass as bass
    import concourse.tile as tile
    from concourse import mybir

    f32 = mybir.dt.float32
    mmdt = getattr(mybir.dt, mm_dtype)
    bf16 = mybir.dt.bfloat16
    u32 = mybir.dt.uint32

    nc = bacc.Bacc("TRN2", target_bir_lowering=False, debug=False)
    qt4 = nc.dram_tensor("qt4", [4, N], mmdt, kind="ExternalInput")
    ct4 = nc.dram_tensor("ct4", [4, N], mmdt, kind="ExternalInput")
    qsq = nc.dram_tensor("qsq", [128, QT], f32, kind="ExternalInput")
    if do_argidx:
        out_idx = nc.dram_tensor("out_idx", [QT, 128, 8], u32, kind="ExternalOutput")
    else:
        out_min = nc.dram_tensor("out_min", [QT, 128, 8], f32, kind="ExternalOutput")

    with tile.TileContext(nc) as tc:
        with ExitStack() as ctx:
            const = ctx.enter_context(tc.tile_pool(name="const", bufs=1))
            strips = ctx.enter_context(tc.tile_pool(name="strip", bufs=strip_bufs))
            psum = ctx.enter_context(
                tc.tile_pool(name="psum", bufs=psum_bufs, space="PSUM"))
            small = ctx.enter_context(tc.tile_pool(name="small", bufs=small_bufs))

            qt4_s = const.tile([4, N], mmdt)
            nc.sync.dma_start(qt4_s[:], qt4.ap())
            ct4_s = const.tile([4, N], mmdt)
            nc.sync.dma_start(ct4_s[:], ct4.ap())
            qsq_s = const.tile([128, QT], f32)
            nc.sync.dma_start(qsq_s[:], qsq.ap())
            zeros8 = const.tile([128, 8], f32)
            nc.vector.memset(zeros8[:], 0.0)

            for t in [tt for _ in range(reps) for tt in range(QT)]:
                strip = strips.tile([128, N], bf16, tag="strip")
                for g in range(4):
                    ps = psum.tile([128, 2048], f32, tag="ps")
                    for j in range(4):
                        nc.tensor.matmul(
                            ps[:, j * 512 : (j + 1) * 512],
                            qt4_s[:, t * 128 : (t + 1) * 128],
                            ct4_s[:, g * 2048 + j * 512 : g * 2048 + (j + 1) * 512],
                            start=True,
                            stop=True,
                        )
                    # d = -s + |q|^2, cast to bf16
                    nc.scalar.activation(
                        strip[:, g * 2048 : (g + 1) * 2048],
                        ps[:],
                        mybir.ActivationFunctionType.Identity,
                        bias=qsq_s[:, t : t + 1],
                        scale=-1.0,
                    )
                if do_accum:
                    # fused: rewrite strip in place (x+0) and min-reduce into dmin
                    dmin = small.tile([128, 1], f32, tag="dmin")
                    nc.vector.tensor_scalar(
                        out=strip[:],
                        in0=strip[:],
                        scalar1=0.0,
                        scalar2=None,
                        op0=mybir.AluOpType.add,
                        op1=mybir.AluOpType.min,
                        accum_out=dmin[:],
                    )
                if do_argidx:
                    # broadcast dmin to [128, 8] bf16 via ACT (scale=0, bias=dmin)
                    min8 = small.tile([128, 8], bf16, tag="min8")
                    nc.scalar.activation(
                        min8[:],
                        zeros8[:],
                        mybir.ActivationFunctionType.Identity,
                        bias=dmin[:],
                        scale=0.0,
                    )
                    idx8 = small.tile([128, 8], u32, tag="idx8")
                    nc.vector.max_index(idx8[:], min8[:], strip[:])
                    nc.sync.dma_start(out_idx.ap()[t], idx8[:])
                elif do_accum:
                    omin = small.tile([128, 8], f32, tag="omin")
                    nc.scalar.activation(
                        omin[:], zeros8[:],
                        mybir.ActivationFunctionType.Identity,
                        bias=dmin[:], scale=0.0,
                    )
                    nc.sync.dma_start(out_min.ap()[t], omin[:])
                else:
                    probe = small.tile([128, 8], f32, tag="omin")
                    sap = bass.AP(strip[:].tensor, strip[:].offset,
                                  [strip[:].ap[0], [1024, 8]])
                    nc.vector.tensor_copy(probe[:], sap)
                    nc.sync.dma_start(out_min.ap()[t], probe[:])

    nc.compile()
    return nc


def _prep_core_inputs(q, c):
    # q, c: [N, 3] float32
    qt4 = np.empty((4, N), np.float32)
    qt4[0:3] = q.T
    qt4[3] = 1.0
    ct4 = np.empty((4, N), np.float32)
    ct4[0:3] = 2.0 * c.T
    csq = np.sum(c.astype(np.float32) * c.astype(np.float32), axis=1)
    ct4[3] = -csq
    qsq_flat = np.sum(q.astype(np.float32) * q.astype(np.float32), axis=1)
    qsq = qsq_flat.reshape(QT, 128).T.copy()
    return {"qt4": qt4, "ct4": ct4, "qsq": qsq}


def _d_row_fp32(q_row, c_all):
    # reference-formula distances of one query row vs all candidates, fp32
    return (
        np.sum(q_row * q_row).astype(np.float32)
        + np.sum(c_all * c_all, axis=1)
        - 2.0 * (c_all @ q_row)
    ).astype(np.float32)


def _indices_from_out(idx8, q, c):
    # idx8: [QT, 128, 8] uint32 -> idx [N] with host tie fixup
    cand = idx8.reshape(N, 8)
    idx = cand[:, 0].astype(np.int64)
    ambiguous = np.where(cand[:, 1] != np.uint32(0xFFFFFFFF))[0]
    for r in ambiguous:
        d = _d_row_fp32(q[r], c)
        idx[r] = int(np.argmin(d))
    return idx


def _loss_one(q, c, idx):
    # mean(1 - exp(-d) * (1/(count+eps))) for one direction (frac terms = 1)
    d = np.sum((q - c[idx]) ** 2, axis=1).astype(np.float32)
    cnt = np.bincount(idx, minlength=N).astype(np.float32)
    w = np.float32(1.0) / (cnt[idx] + np.float32(1e-6))
    return np.mean(np.float32(1.0) - np.exp(-d) * w, dtype=np.float32)


def run_cores(in_maps, trace=False):
    from concourse.bass_utils import run_bass_kernel_spmd

    if "nc" not in _CACHE:
        _CACHE["nc"] = _build()
    nc = _CACHE["nc"]
    res = run_bass_kernel_spmd(
        nc, in_maps, core_ids=list(range(N_CORES)), trace=trace
    )
    return res


def kernel(gts, preds):
    gts = np.ascontiguousarray(np.asarray(gts, dtype=np.float32))
    preds = np.ascontiguousarray(np.asarray(preds, dtype=np.float32))

    qc = []  # per-core (q, c)
    for core in range(N_CORES):
        b, direction = core >> 1, core & 1
        if direction == 0:
            qc.append((gts[b], preds[b]))
        else:
            qc.append((preds[b], gts[b]))

    in_maps = [_prep_core_inputs(q, c) for (q, c) in qc]
    res = run_cores(in_maps)

    loss = np.zeros(B, np.float32)
    per_dir = {}
    for core in range(N_CORES):
        q, c = qc[core]
        idx = _indices_from_out(np.asarray(res.results[core]["out_idx"]), q, c)
        per_dir[core] = _loss_one(q, c, idx)
    for b in range(B):
        loss[b] = (per_dir[2 * b] + per_dir[2 * b + 1]) / np.float32(2.0)
    return loss


# revision 10
# speedup vs baseline: 2.0616x; 1.0724x over previous
"""DensityAwareChamferLoss Trainium2 kernel.

Strategy: 8 cores = (4 batches) x (2 NN directions). Each core runs an
identical SPMD program computing, for 8192 query points against 8192
candidate points, the argmin of squared euclidean distance:

  PE:  s = 2*q.c - |c|^2 at fp32-grade precision but bf16 matmul speed
       (1 cy/row): error-compensated bf16^3 decomposition packed along
       the contraction dim as ONE K=21 bf16 matmul per tile — product
       terms (qh,Ch)(qh,Cl)(ql,Ch)(ql,Cl)(qh,Cm)(qm,Ch) with C=2c plus
       three |c|^2 rows; exact bf16 products accumulate in fp32 PSUM,
       residual ~2^-24 (plain fp32 matmul is 4 cy/row = 874us/core;
       fp32r is tf32-grade and flips ~1.8% of argmins).
  ACT: d = |q|^2 - s  (scale=-1, per-partition bias), cast bf16 -> SBUF
  DVE: fused min-reduce over the [128, 8192] strip (tensor_scalar accum),
       then max_index to recover up to 8 positions matching the min.

Host: bf16 ties (~0.4% of rows) are resolved by recomputing that row's
distances in fp32 (reproduces the reference argmin: 0 flips measured in
numpy simulation and CoreSim); counts/weights/loss are O(N) numpy.

Engine budget per core (HW-calibrated cost model): PE ~250us, ACT
~580us, DVE ~690us (bound: max_index at 1x), total ~845us with overlap
(vs 906us for the fp32-matmul variant).
"""

import sys

if "/opt/trn_rl_repo" not in sys.path:
    sys.path.insert(0, "/opt/trn_rl_repo")

import numpy as np

B = 4
N = 8192
QT = N // 128  # query tiles per core
N_CORES = 8

_CACHE = {}


def _build(mm_dtype="float32", do_accum=True, do_argidx=True, reps=1,
           strip_bufs=2, psum_bufs=2, small_bufs=4, kdim=4):
    from contextlib import ExitStack

    import concourse.bacc as bacc
    import concourse.bass as bass
    import concourse.tile as tile
    from concourse import mybir

    f32 = mybir.dt.float32
    mmdt = getattr(mybir.dt, mm_dtype)
    bf16 = mybir.dt.bfloat16
    u32 = mybir.dt.uint32

    if kdim != 4:
        mmdt = bf16
    nc = bacc.Bacc("TRN2", target_bir_lowering=False, debug=False)
    qt4 = nc.dram_tensor("qt4", [kdim, N], mmdt, kind="ExternalInput")
    ct4 = nc.dram_tensor("ct4", [kdim, N], mmdt, kind="ExternalInput")
    qsq = nc.dram_tensor("qsq", [128, QT], f32, kind="ExternalInput")
    if do_argidx:
        out_idx = nc.dram_tensor("out_idx", [QT, 128, 8], u32, kind="ExternalOutput")
    else:
        out_min = nc.dram_tensor("out_min", [QT, 128, 8], f32, kind="ExternalOutput")

    with tile.TileContext(nc) as tc:
        with ExitStack() as ctx:
            const = ctx.enter_context(tc.tile_pool(name="const", bufs=1))
            strips = ctx.enter_context(tc.tile_pool(name="strip", bufs=strip_bufs))
            psum = ctx.enter_context(
                tc.tile_pool(name="psum", bufs=psum_bufs, space="PSUM"))
            small = ctx.enter_context(tc.tile_pool(name="small", bufs=small_bufs))

            qt4_s = const.tile([kdim, N], mmdt)
            nc.sync.dma_start(qt4_s[:], qt4.ap())
            ct4_s = const.tile([kdim, N], mmdt)
            nc.sync.dma_start(ct4_s[:], ct4.ap())
            qsq_s = const.tile([128, QT], f32)
            nc.sync.dma_start(qsq_s[:], qsq.ap())
            zeros8 = const.tile([128, 8], f32)
            nc.vector.memset(zeros8[:], 0.0)

            for t in [tt for _ in range(reps) for tt in range(QT)]:
                strip = strips.tile([128, N], bf16, tag="strip")
                for g in range(4):
                    ps = psum.tile([128, 2048], f32, tag="ps")
                    for j in range(4):
                        nc.tensor.matmul(
                            ps[:, j * 512 : (j + 1) * 512],
                            qt4_s[:, t * 128 : (t + 1) * 128],
                            ct4_s[:, g * 2048 + j * 512 : g * 2048 + (j + 1) * 512],
                            start=True,
                            stop=True,
                        )
                    # d = -s + |q|^2, cast to bf16
                    nc.scalar.activation(
                        strip[:, g * 2048 : (g + 1) * 2048],
                        ps[:],
                        mybir.ActivationFunctionType.Identity,
                        bias=qsq_s[:, t : t + 1],
                        scale=-1.0,
                    )
                if do_accum:
                    # fused: rewrite strip in place (x+0) and min-reduce into dmin
                    dmin = small.tile([128, 1], f32, tag="dmin")
                    nc.vector.tensor_scalar(
                        out=strip[:],
                        in0=strip[:],
                        scalar1=0.0,
                        scalar2=None,
                        op0=mybir.AluOpType.add,
                        op1=mybir.AluOpType.min,
                        accum_out=dmin[:],
                    )
                if do_argidx:
                    # broadcast dmin to [128, 8] bf16 via ACT (scale=0, bias=dmin)
                    min8 = small.tile([128, 8], bf16, tag="min8")
                    nc.scalar.activation(
                        min8[:],
                        zeros8[:],
                        mybir.ActivationFunctionType.Identity,
                        bias=dmin[:],
                        scale=0.0,
                    )
                    idx8 = small.tile([128, 8], u32, tag="idx8")
                    nc.vector.max_index(idx8[:], min8[:], strip[:])
                    nc.sync.dma_start(out_idx.ap()[t], idx8[:])
                elif do_accum:
                    omin = small.tile([128, 8], f32, tag="omin")
                    nc.scalar.activation(
                        omin[:], zeros8[:],
                        mybir.ActivationFunctionType.Identity,
                        bias=dmin[:], scale=0.0,
                    )
                    nc.sync.dma_start(out_min.ap()[t], omin[:])
                else:
                    probe = small.tile([128, 8], f32, tag="omin")
                    sap = bass.AP(strip[:].tensor, strip[:].offset,
                                  [strip[:].ap[0], [1024, 8]])
                    nc.vector.tensor_copy(probe[:], sap)
                    nc.sync.dma_start(out_min.ap()[t], probe[:])

    nc.compile()
    return nc


def _prep_core_inputs(q, c):
    # q, c: [N, 3] float32
    qt4 = np.empty((4, N), np.float32)
    qt4[0:3] = q.T
    qt4[3] = 1.0
    ct4 = np.empty((4, N), np.float32)
    ct4[0:3] = 2.0 * c.T
    csq = np.sum(c.astype(np.float32) * c.astype(np.float32), axis=1)
    ct4[3] = -csq
    qsq_flat = np.sum(q.astype(np.float32) * q.astype(np.float32), axis=1)
    qsq = qsq_flat.reshape(QT, 128).T.copy()
    return {"qt4": qt4, "ct4": ct4, "qsq": qsq}


def _bf16_split3(x):
    # x (fp32) == hi + lo + mid to ~2^-24 rel; parts exactly bf16
    import ml_dtypes

    bf = ml_dtypes.bfloat16
    hi = x.astype(bf)
    r1 = (x - hi.astype(np.float32)).astype(np.float32)
    lo = r1.astype(bf)
    r2 = (r1 - lo.astype(np.float32)).astype(np.float32)
    mid = r2.astype(bf)
    return hi, lo, mid


def _prep_core_inputs_k21(q, c):
    """Error-compensated bf16^3 decomposition packed along K=21.

    s = sum_k lhsT[k]*rhs[k] = 2q.c - |c|^2 to ~2^-24 relative:
    product terms (qh,Ch),(qh,Cl),(ql,Ch),(ql,Cl),(qh,Cm),(qm,Ch) where
    C = 2c, plus (1,-csq_{h,l,m}). Each part is exactly bf16; PE computes
    exact bf16 x bf16 products accumulated in fp32 PSUM.
    """
    import ml_dtypes

    bf = ml_dtypes.bfloat16
    qh, ql, qm = _bf16_split3(np.ascontiguousarray(q.T, np.float32))  # [3, N]
    Ch, Cl, Cm = _bf16_split3(2.0 * np.ascontiguousarray(c.T, np.float32))
    csq = np.sum(c.astype(np.float32) * c.astype(np.float32), axis=1)
    sh, sl, sm = _bf16_split3(-csq)
    ones = np.ones((1, N), bf)
    qt = np.concatenate(
        [qh, qh, ql, ql, qh, qm, ones, ones, ones], axis=0
    ).astype(bf)
    ct = np.concatenate(
        [Ch, Cl, Ch, Cl, Cm, Ch, sh[None], sl[None], sm[None]], axis=0
    ).astype(bf)
    qsq_flat = np.sum(q.astype(np.float32) * q.astype(np.float32), axis=1)
    qsq = qsq_flat.reshape(QT, 128).T.copy()
    return {"qt4": qt, "ct4": ct, "qsq": qsq}


def _d_row_fp32(q_row, c_all):
    # reference-formula distances of one query row vs all candidates, fp32
    return (
        np.sum(q_row * q_row).astype(np.float32)
        + np.sum(c_all * c_all, axis=1)
        - 2.0 * (c_all @ q_row)
    ).astype(np.float32)


def _indices_from_out(idx8, q, c):
    # idx8: [QT, 128, 8] uint32 -> idx [N] with host tie fixup
    cand = idx8.reshape(N, 8)
    idx = cand[:, 0].astype(np.int64)
    ambiguous = np.where(cand[:, 1] != np.uint32(0xFFFFFFFF))[0]
    for r in ambiguous:
        d = _d_row_fp32(q[r], c)
        idx[r] = int(np.argmin(d))
    return idx


def _loss_one(q, c, idx):
    # mean(1 - exp(-d) * (1/(count+eps))) for one direction (frac terms = 1)
    d = np.sum((q - c[idx]) ** 2, axis=1).astype(np.float32)
    cnt = np.bincount(idx, minlength=N).astype(np.float32)
    w = np.float32(1.0) / (cnt[idx] + np.float32(1e-6))
    return np.mean(np.float32(1.0) - np.exp(-d) * w, dtype=np.float32)


def run_cores(in_maps, trace=False):
    from concourse.bass_utils import run_bass_kernel_spmd

    if "nc" not in _CACHE:
        _CACHE["nc"] = _build(kdim=21)
    nc = _CACHE["nc"]
    res = run_bass_kernel_spmd(
        nc, in_maps, core_ids=list(range(N_CORES)), trace=trace
    )
    return res


def kernel(gts, preds):
    gts = np.ascontiguousarray(np.asarray(gts, dtype=np.float32))
    preds = np.ascontiguousarray(np.asarray(preds, dtype=np.float32))

    qc = []  # per-core (q, c)
    for core in range(N_CORES):
        b, direction = core >> 1, core & 1
        if direction == 0:
            qc.append((gts[b], preds[b]))
        else:
            qc.append((preds[b], gts[b]))

    in_maps = [_prep_core_inputs_k21(q, c) for (q, c) in qc]
    res = run_cores(in_maps)

    loss = np.zeros(B, np.float32)
    per_dir = {}
    for core in range(N_CORES):
        q, c = qc[core]
        idx = _indices_from_out(np.asarray(res.results[core]["out_idx"]), q, c)
        per_dir[core] = _loss_one(q, c, idx)
    for b in range(B):
        loss[b] = (per_dir[2 * b] + per_dir[2 * b + 1]) / np.float32(2.0)
    return loss
